# revision 1
# baseline (speedup 1.0000x reference)
"""Trainium2 Bass kernel for nn_DeepSeekMoE_6777458393401.

Reference computation (B=8, S=2048, IN=512, H=4096, E=8, OUT=512, TOP_K=2):
    h      = x @ Wi^T + bi                      [B,S,H]
    logits = h @ Wr^T + br                      [B,S,E]
    idx    = top_k(softmax(logits), 2)          [B,S,2]   (E=8 experts)
    g      = take_along_axis(h, idx, axis=-1)   [B,S,2]   <- gathers h[...,e]
    a      = mean(g, -1) broadcast over H       [B,S,H]
    out    = a @ Wo^T + bo                      [B,S,OUT]

Because the gather picks *scalar* hidden components h[b,s,e] (e<8) and the
result is broadcast across the whole hidden dim, the module collapses to:

    logits[b,s,:] = x[b,s,:] @ (Wr@Wi)^T + (Wr@bi + br)        (E=8 wide)
    h8[b,s,:]     = x[b,s,:] @ Wi[:8,:]^T + bi[:8]             (8 wide)
    a2[b,s]       = sum of h8 at the top-2 logits              (scalar)
    out[b,s,:]    = a2[b,s] * (0.5*sum_h Wo[:,h]) + bo

i.e. one [B*S,512]@[512,16] GEMM, an 8-wide top-2 select, and a rank-1
outer product. Softmax is monotonic so top-k runs on raw logits.

Sharding: data-parallel over batch, 1 batch element (2048 tokens) per core.
"""

import numpy as np

B, S, IN, H, E, OUT = 8, 2048, 512, 4096, 8, 512
N_CORES = 8
P = 128                 # SBUF partitions
NT = S // P             # 16 token tiles per core
KC = IN // P            # 4 contraction chunks of 128

_CACHE = {}


def _build_nc():
    """Build the per-core Bass program (same NEFF on all 8 cores)."""
    import concourse.bacc as bacc
    import concourse.bass as bass
    import concourse.tile as tile
    from concourse import mybir

    f32 = mybir.dt.float32
    nc = bacc.Bacc("TRN2", target_bir_lowering=False, debug=False)

    # x token-quarter 0 packed with w16=[Wri^T | Wi8^T] -> one full-rate DMA
    xq0w = nc.dram_tensor("xq0w", [P, KC, 512 + 16], f32, kind="ExternalInput")
    xt = nc.dram_tensor("xt", [IN, S - 512], f32, kind="ExternalInput")  # x[b].T cols 512:
    # [c16 (16) | 0.5*Wo.sum(1) (512) | bo (512)] in one row: a single DMA
    # keeps xt quarter1's serial HWDGE pipeline ahead of engine-idle time
    consts = nc.dram_tensor("consts", [1, 16 + 2 * OUT], f32, kind="ExternalInput")
    out = nc.dram_tensor("out", [S, OUT], f32, kind="ExternalOutput")

    with tile.TileContext(nc) as tc:
        with (
            tc.tile_pool(name="singles", bufs=1) as singles,
            tc.tile_pool(name="work", bufs=4) as work,
            tc.tile_pool(name="obuf", bufs=4) as obuf,
            tc.tile_pool(name="psum", bufs=4, space=bass.MemorySpace.PSUM) as psum,
        ):
            # ---- one-time loads -------------------------------------------
            # DMA order: xt quarter0 first (its 2.9us transfer hides the
            # HWDGE/issue pipelines of everything queued behind it), then the
            # small weights, then xt quarters 1-3.
            QT = 4                       # token tiles per quarter
            q = QT * P                   # 512 tokens per quarter
            xq0w_sb = singles.tile([P, KC, q + 16], f32)
            nc.sync.dma_start(out=xq0w_sb[:], in_=xq0w.ap())

            xt_r = xt.ap().rearrange("(k p) t -> p k t", p=P)          # [128,4,1536]
            xt_q = [xq0w_sb]
            for i in range(1, 4):
                xt_q.append(singles.tile([P, KC, q], f32, name=f"xtq{i}", tag=f"xtq{i}"))

            consts_row = singles.tile([1, 16 + 2 * OUT], f32)
            nc.sync.dma_start(out=consts_row[:], in_=consts.ap())
            c_sb = consts_row[0:1, 0:16]
            ones_row = singles.tile([1, P], f32)
            nc.vector.memset(ones_row[:], 1.0)

            # broadcast the const row to 128 partitions on the idle Pool
            # engine (keeps the broadcast off the DMA bandwidth budget)
            cb = singles.tile([P, 16 + 2 * OUT], f32)
            nc.gpsimd.partition_broadcast(cb[:], consts_row[:], channels=P)
            wsum_b = cb[:, 16:16 + OUT]
            bov_b = cb[:, 16 + OUT:16 + 2 * OUT]

            for i in range(1, 4):
                nc.sync.dma_start(out=xt_q[i][:], in_=xt_r[:, :, (i - 1) * q:i * q])

            # ---- per token tile -------------------------------------------
            for grp in range(NT // QT):
                o_sb = obuf.tile([P, QT, OUT], f32)
                for j in range(QT):
                    t = grp * QT + j
                    g_ps = psum.tile([P, 16], f32)
                    # G[tok, 0:8] = logits, G[tok, 8:16] = h8 ; K=512 in 4 chunks
                    for k in range(KC):
                        nc.tensor.matmul(
                            g_ps[:],
                            lhsT=xt_q[grp][:, k, j * P:(j + 1) * P],  # [128K,128tok]
                            rhs=xq0w_sb[:, k, q:q + 16],              # [128K,16]
                            start=(k == 0),
                            stop=False,
                        )
                    # + bias row (K=1 rank-1 update: ones ⊗ c16)
                    nc.tensor.matmul(
                        g_ps[:], lhsT=ones_row[:], rhs=c_sb[:], start=False, stop=True,
                    )

                    g_sb = work.tile([P, 16], f32)
                    nc.scalar.copy(out=g_sb[:], in_=g_ps[:])

                    # top-8 sort of the 8 logits -> 2nd largest at column 1
                    top8 = work.tile([P, 8], f32)
                    nc.vector.max(out=top8[:], in_=g_sb[:, 0:8])

                    # a2 = sum over experts of (logit >= m2) * h8  (= top-2 sum)
                    junk8 = work.tile([P, 8], f32)
                    a2 = work.tile([P, 1], f32)
                    nc.vector.scalar_tensor_tensor(
                        out=junk8[:],
                        in0=g_sb[:, 0:8],
                        scalar=top8[:, 1:2],
                        in1=g_sb[:, 8:16],
                        op0=mybir.AluOpType.is_ge,
                        op1=mybir.AluOpType.mult,
                        accum_out=a2[:],
                    )

                    # out[tok,:] = a2 * (0.5*WoSum) + bo
                    nc.vector.scalar_tensor_tensor(
                        out=o_sb[:, j, :],
                        in0=wsum_b[:],
                        scalar=a2[:],
                        in1=bov_b[:],
                        op0=mybir.AluOpType.mult,
                        op1=mybir.AluOpType.add,
                    )
                # one 1MB DMA per 4 token tiles: out rows [grp*512, (grp+1)*512)
                nc.sync.dma_start(
                    out=out.ap().rearrange("(g j p) o -> p (g j) o", p=P, j=QT)[
                        :, grp * QT:(grp + 1) * QT, :
                    ],
                    in_=o_sb[:],
                )

    # Drop the framework preamble's const-tile memsets: the bir verifier
    # confirms nothing in this program reads const-* tiles, and they make
    # Pool the last engine into the entry barrier (~0.4us of startup).
    for bb in nc.main_func.blocks:
        dead = [
            i for i in bb.instructions
            if type(i).__name__ == "InstMemset" and "const-" in str(i.outs[0])
        ]
        for ins in dead:
            bb.instructions.remove(ins)

    nc.compile()
    return nc


def _prep_inputs(x, Wi, bi, Wr, br, Wo, bo):
    """Fold weights on host (tiny: ~17 MFLOP) and build per-core in_maps."""
    f32 = np.float32
    x = np.asarray(x, f32)
    Wi = np.asarray(Wi, f32)
    bi = np.asarray(bi, f32)
    Wr = np.asarray(Wr, f32)
    br = np.asarray(br, f32)
    Wo = np.asarray(Wo, f32)
    bo = np.asarray(bo, f32)

    Wri = (Wr.astype(np.float64) @ Wi.astype(np.float64)).astype(f32)   # [E, IN]
    cr = (Wr.astype(np.float64) @ bi.astype(np.float64)).astype(f32) + br
    w16 = np.empty((IN, 16), f32)
    w16[:, 0:8] = Wri.T
    w16[:, 8:16] = Wi[0:8, :].T
    w16_pkj = w16.reshape(KC, P, 16).transpose(1, 0, 2)                 # [p,k,j]
    c16 = np.concatenate([cr, bi[0:8]]).astype(f32)
    wsum = (0.5 * Wo.sum(axis=1, dtype=np.float64)).astype(f32)
    consts = np.concatenate([c16, wsum, bo.astype(f32)]).reshape(1, 16 + 2 * OUT)

    shared = {"consts": consts}
    in_maps = []
    for b in range(N_CORES):
        m = dict(shared)
        xtb = x[b].T                                                    # [512, 2048]
        xq0w = np.empty((P, KC, 512 + 16), f32)
        xq0w[:, :, :512] = xtb.reshape(KC, P, S)[:, :, 0:512].transpose(1, 0, 2)
        xq0w[:, :, 512:] = w16_pkj
        m["xq0w"] = xq0w
        m["xt"] = np.ascontiguousarray(xtb[:, 512:])
        in_maps.append(m)
    return in_maps


def run(inputs, trace=False, **run_kwargs):
    """Compile (cached), run on 8 cores, gather. Returns (out, BassKernelResults)."""
    from concourse.bass_utils import run_bass_kernel_spmd

    if "nc" not in _CACHE:
        _CACHE["nc"] = _build_nc()
    nc = _CACHE["nc"]

    in_maps = _prep_inputs(**inputs)
    try:
        res = run_bass_kernel_spmd(
            nc, in_maps, core_ids=list(range(N_CORES)), trace=trace, **run_kwargs
        )
    except Exception:
        # one retry for transient device wedges (NRT_TIMEOUT / unrecoverable)
        import time

        time.sleep(10)
        res = run_bass_kernel_spmd(
            nc, in_maps, core_ids=list(range(N_CORES)), trace=trace, **run_kwargs
        )
    out = np.stack([r["out"] for r in res.results], axis=0)  # [B, S, OUT]
    return out, res


def kernel(x, Wi, bi, Wr, br, Wo, bo) -> np.ndarray:
    out, _ = run(dict(x=x, Wi=Wi, bi=bi, Wr=Wr, br=br, Wo=Wo, bo=bo))
    return out



# revision 3
# speedup vs baseline: 1.3255x; 1.3255x over previous
"""Trainium2 Bass kernel for nn_DeepSeekMoE_6777458393401.

Reference computation (B=8, S=2048, IN=512, H=4096, E=8, OUT=512, TOP_K=2):
    h      = x @ Wi^T + bi                      [B,S,H]
    logits = h @ Wr^T + br                      [B,S,E]
    idx    = top_k(softmax(logits), 2)          [B,S,2]   (E=8 experts)
    g      = take_along_axis(h, idx, axis=-1)   [B,S,2]   <- gathers h[...,e]
    a      = mean(g, -1) broadcast over H       [B,S,H]
    out    = a @ Wo^T + bo                      [B,S,OUT]

Because the gather picks *scalar* hidden components h[b,s,e] (e<8) and the
result is broadcast across the whole hidden dim, the module collapses to:

    logits[b,s,:] = x[b,s,:] @ (Wr@Wi)^T + (Wr@bi + br)        (E=8 wide)
    h8[b,s,:]     = x[b,s,:] @ Wi[:8,:]^T + bi[:8]             (8 wide)
    a2[b,s]       = sum of h8 at the top-2 logits              (scalar)
    out[b,s,:]    = a2[b,s] * (0.5*sum_h Wo[:,h]) + bo

i.e. one [B*S,512]@[512,16] GEMM, an 8-wide top-2 select, and a rank-1
outer product. Softmax is monotonic so top-k runs on raw logits.

The kernel is DMA-bound (in+out streamed at the 360 GB/s aggregate), so the
wire traffic is quantized: x travels as fp16 plus an fp8(e4m3) residual
(x = x_hi + x_lo*2^-11, ~19 effective mantissa bits so the top-2 selection
matches the f32 reference on all but ~1e-5 of tokens), and the output is
written as fp16. The 512x16 folded weight travels as fp16 hi + fp16 lo
(*2^10) + fp8 (*2^5) so weight quantization never limits logit accuracy.

Per 128-token tile, three PSUM accumulators are built on the PE:
    AB[:, 0:16] = x_hi @ w_hi     AB[:, 16:32] = x_hi @ w_lo*2^10
    C           = (x_lo*2^11) @ (w*2^5 as fp8)
and combined on DVE:  G = AB[:,0:16] + 2^-10*AB[:,16:32] + 2^-16*C + c16.

Sharding: data-parallel over batch, 1 batch element (2048 tokens) per core.
"""

import numpy as np

B, S, IN, H, E, OUT = 8, 2048, 512, 4096, 8, 512
N_CORES = 8
P = 128                 # SBUF partitions
NT = S // P             # 16 token tiles per core
KC = IN // P            # 4 contraction chunks of 128
QT = 4                  # token tiles per group (512 tokens)
NG = NT // QT           # 4 groups
Q = QT * P              # tokens per group

R_SHIFT = 2.0 ** 11     # x residual stored as e4m3 of (x - fp16(x)) * 2^11
WLO_SHIFT = 2.0 ** 10   # w residual stored as fp16 of (w - fp16(w)) * 2^10
W8_SHIFT = 2.0 ** 5     # fp8 copy of w stored as e4m3 of w * 2^5

_CACHE = {}


def _build_nc(bo_is_zero):
    """Build the per-core Bass program (same NEFF on all 8 cores)."""
    import concourse.bacc as bacc
    import concourse.bass as bass
    import concourse.tile as tile
    from concourse import mybir

    f32 = mybir.dt.float32
    f16 = mybir.dt.float16
    f8 = mybir.dt.float8e4
    nc = bacc.Bacc("TRN2", target_bir_lowering=False, debug=False)

    # x_hi quarter 0 packed with w_hi|w_lo -> one full-rate DMA
    xhw = nc.dram_tensor("xhw", [P, KC, Q + 32], f16, kind="ExternalInput")
    xh = nc.dram_tensor("xh", [P, KC, S - Q], f16, kind="ExternalInput")
    # x_lo quarter 0 packed with w8
    xlw = nc.dram_tensor("xlw", [P, KC, Q + 16], f8, kind="ExternalInput")
    xl = nc.dram_tensor("xl", [P, KC, S - Q], f8, kind="ExternalInput")
    # [c16 x4 (64) | 0.5*Wo.sum(1) (512) | bo (512)] in one row
    consts = nc.dram_tensor("consts", [1, 64 + 2 * OUT], f32, kind="ExternalInput")
    out = nc.dram_tensor("out", [S, OUT], f16, kind="ExternalOutput")

    with tile.TileContext(nc) as tc:
        with (
            tc.tile_pool(name="singles", bufs=1) as singles,
            tc.tile_pool(name="work", bufs=4) as work,
            tc.tile_pool(name="obuf", bufs=4) as obuf,
            tc.tile_pool(name="psum", bufs=4, space=bass.MemorySpace.PSUM) as psum,
        ):
            # ---- one-time loads -------------------------------------------
            # DMA order: the big packed quarter-0 transfers first (their
            # ~2.3us hides the HWDGE/issue pipelines of everything queued
            # behind), then consts, then the remaining quarters interleaved
            # hi/lo so each group's operands arrive together.
            xhw_sb = singles.tile([P, KC, Q + 32], f16)
            nc.sync.dma_start(out=xhw_sb[:], in_=xhw.ap())
            xlw_sb = singles.tile([P, KC, Q + 16], f8)
            nc.sync.dma_start(out=xlw_sb[:], in_=xlw.ap())

            consts_row = singles.tile([1, 64 + 2 * OUT], f32)
            nc.sync.dma_start(out=consts_row[:], in_=consts.ap())

            xh_q = [xhw_sb]
            xl_q = [xlw_sb]
            for g in range(1, NG):
                xh_q.append(singles.tile([P, KC, Q], f16, name=f"xhq{g}", tag=f"xhq{g}"))
                xl_q.append(singles.tile([P, KC, Q], f8, name=f"xlq{g}", tag=f"xlq{g}"))
                nc.sync.dma_start(
                    out=xh_q[g][:], in_=xh.ap()[:, :, (g - 1) * Q:g * Q]
                )
                nc.sync.dma_start(
                    out=xl_q[g][:], in_=xl.ap()[:, :, (g - 1) * Q:g * Q]
                )

            # broadcast const rows to 128 partitions on the idle Pool engine
            cb16 = singles.tile([P, QT, 16], f32)   # c16 replicated x4
            nc.gpsimd.partition_broadcast(cb16[:], consts_row[0:1, 0:64], channels=P)
            cbw = singles.tile([P, 2, OUT], f32)    # [wsum | bo]
            nc.gpsimd.partition_broadcast(
                cbw[:], consts_row[0:1, 64:64 + 2 * OUT], channels=P
            )

            # ---- per token-tile group -------------------------------------
            for g in range(NG):
                ab_ps = psum.tile([P, QT, 32], f32)
                c_ps = psum.tile([P, QT, 16], f32)
                for j in range(QT):
                    for k in range(KC):
                        nc.tensor.matmul(
                            ab_ps[:, j, :],
                            lhsT=xh_q[g][:, k, j * P:(j + 1) * P],   # [128K,128tok]
                            rhs=xhw_sb[:, k, Q:Q + 32],              # [128K,32]
                            start=(k == 0),
                            stop=(k == KC - 1),
                        )
                    for k in range(KC):
                        nc.tensor.matmul(
                            c_ps[:, j, :],
                            lhsT=xl_q[g][:, k, j * P:(j + 1) * P],
                            rhs=xlw_sb[:, k, Q:Q + 16],              # [128K,16]
                            start=(k == 0),
                            stop=(k == KC - 1),
                        )

                # G = AB[:,:,0:16] + 2^-10*AB[:,:,16:32] + 2^-16*C + c16
                tb = work.tile([P, QT, 16], f32)
                nc.vector.scalar_tensor_tensor(
                    out=tb[:],
                    in0=ab_ps[:, :, 16:32],
                    scalar=1.0 / WLO_SHIFT,
                    in1=cb16[:],
                    op0=mybir.AluOpType.mult,
                    op1=mybir.AluOpType.add,
                )
                tg = work.tile([P, QT, 16], f32)
                nc.vector.scalar_tensor_tensor(
                    out=tg[:],
                    in0=c_ps[:],
                    scalar=1.0 / (R_SHIFT * W8_SHIFT),
                    in1=tb[:],
                    op0=mybir.AluOpType.mult,
                    op1=mybir.AluOpType.add,
                )
                gt = work.tile([P, QT, 16], f32)
                nc.vector.tensor_tensor(
                    gt[:], ab_ps[:, :, 0:16], tg[:], mybir.AluOpType.add
                )

                o_sb = obuf.tile([P, QT, OUT], f16)
                for j in range(QT):
                    # top-8 sort of the 8 logits -> 2nd largest at column 1
                    top8 = work.tile([P, 8], f32)
                    nc.vector.max(out=top8[:], in_=gt[:, j, 0:8])

                    # a2 = sum over experts of (logit >= m2) * h8  (top-2 sum)
                    junk8 = work.tile([P, 8], f32)
                    a2 = work.tile([P, 1], f32)
                    nc.vector.scalar_tensor_tensor(
                        out=junk8[:],
                        in0=gt[:, j, 0:8],
                        scalar=top8[:, 1:2],
                        in1=gt[:, j, 8:16],
                        op0=mybir.AluOpType.is_ge,
                        op1=mybir.AluOpType.mult,
                        accum_out=a2[:],
                    )

                    # out[tok,:] = a2 * (0.5*WoSum) + bo, spread across DVE
                    # and the otherwise-idle Activation engine (Pool rejects
                    # TensorScalarPtr at codegen).
                    if bo_is_zero and j >= 1:
                        nc.scalar.mul(o_sb[:, j, :], cbw[:, 0, :], a2[:])
                    else:
                        nc.vector.scalar_tensor_tensor(
                            out=o_sb[:, j, :],
                            in0=cbw[:, 0, :],
                            scalar=a2[:],
                            in1=cbw[:, 1, :],
                            op0=mybir.AluOpType.mult,
                            op1=mybir.AluOpType.add,
                        )
                # one 512KB DMA per group: out rows [g*512, (g+1)*512)
                nc.sync.dma_start(
                    out=out.ap().rearrange("(g j p) o -> p (g j) o", p=P, j=QT)[
                        :, g * QT:(g + 1) * QT, :
                    ],
                    in_=o_sb[:],
                )

    # Drop the framework preamble's const-tile memsets: nothing in this
    # program reads const-* tiles, and they make Pool the last engine into
    # the entry barrier (~0.4us of startup).
    for bb in nc.main_func.blocks:
        dead = [
            i for i in bb.instructions
            if type(i).__name__ == "InstMemset" and "const-" in str(i.outs[0])
        ]
        for ins in dead:
            bb.instructions.remove(ins)

    nc.compile()
    return nc


def _prep_inputs(x, Wi, bi, Wr, br, Wo, bo):
    """Fold weights on host (tiny: ~17 MFLOP) and build per-core in_maps."""
    import ml_dtypes

    f32, f16, f64 = np.float32, np.float16, np.float64
    e4m3 = ml_dtypes.float8_e4m3
    x = np.asarray(x, f32)
    Wi = np.asarray(Wi, f32)
    bi = np.asarray(bi, f32)
    Wr = np.asarray(Wr, f32)
    br = np.asarray(br, f32)
    Wo = np.asarray(Wo, f32)
    bo = np.asarray(bo, f32)

    Wri = (Wr.astype(f64) @ Wi.astype(f64)).astype(f32)                 # [E, IN]
    cr = (Wr.astype(f64) @ bi.astype(f64)).astype(f32) + br
    w16 = np.empty((IN, 16), f32)
    w16[:, 0:8] = Wri.T
    w16[:, 8:16] = Wi[0:8, :].T
    w_hi = w16.astype(f16)
    w_lo = ((w16.astype(f64) - w_hi.astype(f64)) * WLO_SHIFT).astype(f32).astype(f16)
    w8 = (w16 * np.float32(W8_SHIFT)).astype(e4m3)

    def pkj(a):                                                          # [IN,16]->[p,k,16]
        return np.ascontiguousarray(a.reshape(KC, P, 16).transpose(1, 0, 2))

    c16 = np.concatenate([cr, bi[0:8]]).astype(f32)
    wsum = (0.5 * Wo.sum(axis=1, dtype=f64)).astype(f32)
    consts = np.concatenate([np.tile(c16, QT), wsum, bo]).reshape(1, 64 + 2 * OUT)

    shared = {"consts": consts}
    in_maps = []
    for b in range(N_CORES):
        m = dict(shared)
        # token-major -> [p, k, t] with contraction index i = k*128+p
        pkt = np.ascontiguousarray(
            x[b].T.reshape(KC, P, S).transpose(1, 0, 2)
        )                                                                # [128,4,2048]
        x_hi = pkt.astype(f16)
        x_lo = (
            (pkt.astype(f64) - x_hi.astype(f64)) * R_SHIFT
        ).astype(f32).astype(e4m3)

        xhw = np.empty((P, KC, Q + 32), f16)
        xhw[:, :, :Q] = x_hi[:, :, 0:Q]
        xhw[:, :, Q:Q + 16] = pkj(w_hi)
        xhw[:, :, Q + 16:] = pkj(w_lo)
        m["xhw"] = xhw
        m["xh"] = np.ascontiguousarray(x_hi[:, :, Q:])

        xlw = np.empty((P, KC, Q + 16), e4m3)
        xlw[:, :, :Q] = x_lo[:, :, 0:Q]
        xlw[:, :, Q:] = pkj(w8)
        m["xlw"] = xlw
        m["xl"] = np.ascontiguousarray(x_lo[:, :, Q:])
        in_maps.append(m)
    return in_maps, bool(np.all(bo == 0.0))


def run(inputs, trace=False, **run_kwargs):
    """Compile (cached), run on 8 cores, gather. Returns (out, BassKernelResults)."""
    from concourse.bass_utils import run_bass_kernel_spmd

    in_maps, bo_is_zero = _prep_inputs(**inputs)
    key = ("nc", bo_is_zero)
    if key not in _CACHE:
        _CACHE[key] = _build_nc(bo_is_zero)
        _CACHE["nc"] = _CACHE[key]  # for test.py's TimelineSim hook
    nc = _CACHE[key]
    _CACHE["nc"] = nc

    try:
        res = run_bass_kernel_spmd(
            nc, in_maps, core_ids=list(range(N_CORES)), trace=trace, **run_kwargs
        )
    except Exception:
        # one retry for transient device wedges (NRT_TIMEOUT / unrecoverable)
        import time

        time.sleep(10)
        res = run_bass_kernel_spmd(
            nc, in_maps, core_ids=list(range(N_CORES)), trace=trace, **run_kwargs
        )
    out = np.stack([r["out"] for r in res.results], axis=0).astype(np.float32)
    return out, res


def kernel(x, Wi, bi, Wr, br, Wo, bo) -> np.ndarray:
    out, _ = run(dict(x=x, Wi=Wi, bi=bi, Wr=Wr, br=br, Wo=Wo, bo=bo))
    return out


# revision 29
# speedup vs baseline: 1.4552x; 1.0978x over previous
"""Trainium2 Bass kernel for nn_DeepSeekMoE_6777458393401.

Reference computation (B=8, S=2048, IN=512, H=4096, E=8, OUT=512, TOP_K=2):
    h      = x @ Wi^T + bi                      [B,S,H]
    logits = h @ Wr^T + br                      [B,S,E]
    idx    = top_k(softmax(logits), 2)          [B,S,2]   (E=8 experts)
    g      = take_along_axis(h, idx, axis=-1)   [B,S,2]   <- gathers h[...,e]
    a      = mean(g, -1) broadcast over H       [B,S,H]
    out    = a @ Wo^T + bo                      [B,S,OUT]

Because the gather picks *scalar* hidden components h[b,s,e] (e<8) and the
result is broadcast across the whole hidden dim, the module collapses to:

    logits[b,s,:] = x[b,s,:] @ (Wr@Wi)^T + (Wr@bi + br)        (E=8 wide)
    h8[b,s,:]     = x[b,s,:] @ Wi[:8,:]^T + bi[:8]             (8 wide)
    a2[b,s]       = sum of h8 at the top-2 logits              (scalar)
    out[b,s,:]    = a2[b,s] * (0.5*sum_h Wo[:,h]) + bo

i.e. one [B*S,512]@[512,16] GEMM, an 8-wide top-2 select, and a rank-1
outer product. Softmax is monotonic so top-k runs on raw logits.

The kernel is DMA-bound (in+out streamed through the 360 GB/s aggregate DMA
path), so wire traffic is quantized: x travels as fp16 plus an fp8(e4m3)
residual (x = x_hi + x_lo*2^-11, ~19 effective mantissa bits so the top-2
selection matches the f32 reference on all but ~1e-5 of tokens), and the
output is written as fp16. The 512x16 folded weight travels as fp16 hi +
fp16 lo (*2^10) + fp8 (*2^5) so weight quantization never limits accuracy.

Per 128-token tile, three PSUM accumulators are built on the PE:
    AB[:, 0:16] = x_hi @ w_hi     AB[:, 16:32] = x_hi @ w_lo*2^10
    C           = (x_lo*2^11) @ (w*2^5 as fp8)
combined on DVE as  G = (C*2^-6 + AB[:,16:32])*2^-10 + AB[:,0:16]  (+c16).

The output expansion (a2 x wsum, 512 fp16 per token) is spread over Pool,
Activation and DVE so no single engine paces the group pipeline, and the
output leaves in 8 half-group DMAs so transfers start as soon as two token
tiles are finished.

Sharding: data-parallel over batch, 1 batch element (2048 tokens) per core.
"""

import numpy as np

B, S, IN, H, E, OUT = 8, 2048, 512, 4096, 8, 512
N_CORES = 8
P = 128                 # SBUF partitions
NT = S // P             # 16 token tiles per core
KC = IN // P            # 4 contraction chunks of 128
QT = 4                  # token tiles per group (512 tokens)
NG = NT // QT           # 4 groups
Q = QT * P              # tokens per group

R_SHIFT = 2.0 ** 11     # x residual stored as e4m3 of (x - fp16(x)) * 2^11
WLO_SHIFT = 2.0 ** 16   # w residual stored as fp16 of (w - fp16(w)) * 2^16
W8_SHIFT = 2.0 ** 5     # fp8 copy of w stored as e4m3 of w * 2^5
CSCALE = 1.0 / (R_SHIFT * W8_SHIFT)   # = 2^-16 = 1/WLO_SHIFT too

USE_POOL_TT = True      # rank-1 expansion of tile 0 on the Pool engine

_CACHE = {}


def _build_nc(bo_is_zero, c16_is_zero):
    """Build the per-core Bass program (same NEFF on all 8 cores)."""
    import concourse.bacc as bacc
    import concourse.bass as bass
    import concourse.tile as tile
    from concourse import mybir

    f32 = mybir.dt.float32
    f16 = mybir.dt.float16
    f8 = mybir.dt.float8e4
    nc = bacc.Bacc("TRN2", target_bir_lowering=False, debug=False)

    # x quarter 0 packed with the weights -> big full-rate first DMAs whose
    # transfer time hides the HWDGE/issue pipeline of everything behind them
    xhw = nc.dram_tensor("xhw", [P, KC, Q + 32], f16, kind="ExternalInput")
    xlw = nc.dram_tensor("xlw", [P, KC, Q + 16], f8, kind="ExternalInput")
    xh = nc.dram_tensor("xh", [P, KC, S - Q], f16, kind="ExternalInput")
    xl = nc.dram_tensor("xl", [P, KC, S - Q], f8, kind="ExternalInput")
    # [c16 x4 (64) | 0.5*Wo.sum(1) (512) | bo (512)] in one row
    consts = nc.dram_tensor("consts", [1, 64 + 2 * OUT], f32, kind="ExternalInput")
    out = nc.dram_tensor("out", [S, OUT], f16, kind="ExternalOutput")
    out_r = out.ap().rearrange("(t p) o -> p t o", p=P)          # [128,16,512]

    with tile.TileContext(nc) as tc:
        with (
            tc.tile_pool(name="singles", bufs=1) as singles,
            tc.tile_pool(name="work", bufs=4) as work,
            tc.tile_pool(name="obuf", bufs=4) as obuf,
            tc.tile_pool(name="psum", bufs=4, space=bass.MemorySpace.PSUM) as psum,
        ):
            # ---- one-time loads -------------------------------------------
            # Packed quarter-0 transfers first, then consts, then the other
            # x quarters hi/lo interleaved so each group's operands arrive
            # together.
            xhw_sb = singles.tile([P, KC, Q + 32], f16)
            nc.sync.dma_start(out=xhw_sb[:], in_=xhw.ap())
            xlw_sb = singles.tile([P, KC, Q + 16], f8)
            nc.sync.dma_start(out=xlw_sb[:], in_=xlw.ap())
            consts_row = singles.tile([1, 64 + 2 * OUT], f32)
            nc.sync.dma_start(out=consts_row[:], in_=consts.ap())

            whl_sb = xhw_sb[:, :, Q:Q + 32]
            w8_sb = xlw_sb[:, :, Q:Q + 16]
            # quarters 1-2 as 4-tile groups; quarter 3 split into two 2-tile
            # groups (fp16 halves are separate DMAs, the fp8 half stays one
            # DMA to keep descriptors >= 512B) so the final output chunks
            # clear the pipeline before their DMA slots.
            xh_q, xl_q = [xhw_sb], [xlw_sb]
            for g in (1, 2):
                xh_q.append(singles.tile([P, KC, Q], f16, name=f"xhq{g}", tag=f"xhq{g}"))
                xl_q.append(singles.tile([P, KC, Q], f8, name=f"xlq{g}", tag=f"xlq{g}"))
                nc.sync.dma_start(
                    out=xh_q[g][:], in_=xh.ap()[:, :, (g - 1) * Q:g * Q]
                )
                nc.sync.dma_start(
                    out=xl_q[g][:], in_=xl.ap()[:, :, (g - 1) * Q:g * Q]
                )
            xh_q3a = singles.tile([P, KC, Q // 2], f16, name="xhq3a", tag="xhq3a")
            xh_q3b = singles.tile([P, KC, Q // 2], f16, name="xhq3b", tag="xhq3b")
            xl_q3 = singles.tile([P, KC, Q], f8, name="xlq3", tag="xlq3")
            nc.sync.dma_start(out=xh_q3a[:], in_=xh.ap()[:, :, 2 * Q:2 * Q + Q // 2])
            nc.sync.dma_start(out=xl_q3[:], in_=xl.ap()[:, :, 2 * Q:3 * Q])
            nc.sync.dma_start(out=xh_q3b[:], in_=xh.ap()[:, :, 2 * Q + Q // 2:3 * Q])

            # broadcast const rows to 128 partitions on the idle Pool engine
            cbw = singles.tile([P, 2, OUT], f32)    # [wsum | bo]
            nc.gpsimd.partition_broadcast(
                cbw[:], consts_row[0:1, 64:64 + 2 * OUT], channels=P
            )
            if not c16_is_zero:
                cb16 = singles.tile([P, QT, 16], f32)   # c16 replicated x4
                nc.gpsimd.partition_broadcast(
                    cb16[:], consts_row[0:1, 0:64], channels=P
                )

            # ---- per token-tile group -------------------------------------
            # (xh slice source, xl slice source, xl col offset, base tile, n tiles)
            group_specs = [
                (xh_q[0], xl_q[0], 0, 0, QT),
                (xh_q[1], xl_q[1], 0, 4, QT),
                (xh_q[2], xl_q[2], 0, 8, QT),
                (xh_q3a, xl_q3, 0, 12, 2),
                (xh_q3b, xl_q3, Q // 2, 14, 2),
            ]
            for g, (xh_g, xl_g, lo_off, tbase, nt) in enumerate(group_specs):
                a_ps = psum.tile([P, nt, 16], f32)
                c_ps = psum.tile([P, nt, 16], f32)
                for j in range(nt):
                    t0 = j * P
                    for k in range(KC):
                        nc.tensor.matmul(
                            a_ps[:, j, :],
                            lhsT=xh_g[:, k, t0:t0 + P],          # [128K,128tok]
                            rhs=whl_sb[:, k, 0:16],              # [128K,16]
                            start=(k == 0),
                            stop=(k == KC - 1),
                        )
                    # correction accumulator: x_lo@w8 and x_hi@w_lo share the
                    # 2^16 scale, so both fold into one PSUM region
                    for k in range(KC):
                        nc.tensor.matmul(
                            c_ps[:, j, :],
                            lhsT=xl_g[:, k, lo_off + t0:lo_off + t0 + P],
                            rhs=w8_sb[:, k, :],                  # [128K,16]
                            start=(k == 0),
                            stop=False,
                        )
                    for k in range(KC):
                        nc.tensor.matmul(
                            c_ps[:, j, :],
                            lhsT=xh_g[:, k, t0:t0 + P],
                            rhs=whl_sb[:, k, 16:32],             # [128K,16]
                            start=False,
                            stop=(k == KC - 1),
                        )

                # G = A + 2^-16*C + c16   (each op reads one PSUM operand;
                # both stay on DVE — routing one through ACT chains the next
                # group's combine behind this group's ACT expansions)
                tb = work.tile([P, nt, 16], f32)
                if c16_is_zero:
                    nc.vector.tensor_scalar_mul(tb[:], c_ps[:], CSCALE)
                else:
                    nc.vector.scalar_tensor_tensor(
                        out=tb[:],
                        in0=c_ps[:],
                        scalar=CSCALE,
                        in1=cb16[:, 0:nt, :],
                        op0=mybir.AluOpType.mult,
                        op1=mybir.AluOpType.add,
                    )
                gt = work.tile([P, nt, 16], f32)
                nc.vector.tensor_tensor(
                    gt[:], a_ps[:], tb[:], mybir.AluOpType.add
                )

                # top-2 select per token tile
                a2s = []
                for j in range(nt):
                    top8 = work.tile([P, 8], f32)
                    nc.vector.max(out=top8[:], in_=gt[:, j, 0:8])
                    junk8 = work.tile([P, 8], f32)
                    a2 = work.tile([P, 1], f32)
                    nc.vector.scalar_tensor_tensor(
                        out=junk8[:],
                        in0=gt[:, j, 0:8],
                        scalar=top8[:, 1:2],
                        in1=gt[:, j, 8:16],
                        op0=mybir.AluOpType.is_ge,
                        op1=mybir.AluOpType.mult,
                        accum_out=a2[:],
                    )
                    a2s.append(a2)

                # out[tok,:] = a2 * (0.5*WoSum) + bo. Expansion is spread
                # over Pool/ACT/DVE; the last two tiles go to the fastest
                # lanes so each half-group output DMA leaves promptly.
                o_sb = obuf.tile([P, nt, OUT], f16)

                def expand(j, lane, c0=0, c1=OUT):
                    if lane == "pool+dve":
                        expand(j, "pool", 0, OUT // 2)
                        expand(j, "dve", OUT // 2, OUT)
                    elif lane == "pool":
                        nc.gpsimd.tensor_tensor(
                            o_sb[:, j, c0:c1],
                            cbw[:, 0, c0:c1],
                            a2s[j][:].to_broadcast((P, c1 - c0)),
                            mybir.AluOpType.mult,
                        )
                    elif lane == "act":
                        nc.scalar.mul(o_sb[:, j, c0:c1], cbw[:, 0, c0:c1], a2s[j][:])
                    else:
                        nc.vector.scalar_tensor_tensor(
                            out=o_sb[:, j, c0:c1],
                            in0=cbw[:, 0, c0:c1],
                            scalar=a2s[j][:],
                            in1=cbw[:, 1, c0:c1],
                            op0=mybir.AluOpType.mult,
                            op1=mybir.AluOpType.add,
                        )

                if not bo_is_zero:
                    lanes = ["dve"] * nt
                elif nt == QT:
                    lanes = ["pool" if USE_POOL_TT else "dve", "act", "dve", "act"]
                elif g == len(group_specs) - 2:
                    # first tail group: its DMA slot is late, so the slow Pool
                    # lane is free capacity here and keeps DVE clear for the
                    # final group's chain
                    lanes = ["pool" if USE_POOL_TT else "dve", "act"]
                else:
                    # final group: fastest lanes in parallel
                    lanes = ["dve", "act"]
                for half in range(nt // 2):
                    for j in (2 * half, 2 * half + 1):
                        expand(j, lanes[j])
                    nc.sync.dma_start(
                        out=out_r[:, tbase + 2 * half:tbase + 2 * half + 2, :],
                        in_=o_sb[:, 2 * half:2 * half + 2, :],
                    )

    # Drop the framework preamble's const-tile memsets: nothing in this
    # program reads const-* tiles, and they make Pool the last engine into
    # the entry barrier (~0.4us of startup).
    for bb in nc.main_func.blocks:
        dead = [
            i for i in bb.instructions
            if type(i).__name__ == "InstMemset" and "const-" in str(i.outs[0])
        ]
        for ins in dead:
            bb.instructions.remove(ins)

    nc.compile()
    return nc


def _prep_inputs(x, Wi, bi, Wr, br, Wo, bo):
    """Fold weights on host (tiny: ~17 MFLOP) and build per-core in_maps."""
    import ml_dtypes

    f32, f16, f64 = np.float32, np.float16, np.float64
    e4m3 = ml_dtypes.float8_e4m3
    x = np.asarray(x, f32)
    Wi = np.asarray(Wi, f32)
    bi = np.asarray(bi, f32)
    Wr = np.asarray(Wr, f32)
    br = np.asarray(br, f32)
    Wo = np.asarray(Wo, f32)
    bo = np.asarray(bo, f32)

    Wri = (Wr.astype(f64) @ Wi.astype(f64)).astype(f32)                 # [E, IN]
    cr = (Wr.astype(f64) @ bi.astype(f64)).astype(f32) + br
    w16 = np.empty((IN, 16), f32)
    w16[:, 0:8] = Wri.T
    w16[:, 8:16] = Wi[0:8, :].T
    w_hi = w16.astype(f16)
    w_lo = ((w16.astype(f64) - w_hi.astype(f64)) * WLO_SHIFT).astype(f32).astype(f16)
    w8 = (w16 * np.float32(W8_SHIFT)).astype(e4m3)

    def pkj(a, w):                                                       # [IN,w]->[p,k,w]
        return np.ascontiguousarray(a.reshape(KC, P, w).transpose(1, 0, 2))

    whl = np.concatenate([pkj(w_hi, 16), pkj(w_lo, 16)], axis=2)         # [p,k,32]
    w8_pkj = pkj(w8, 16)
    c16 = np.concatenate([cr, bi[0:8]]).astype(f32)
    wsum = (0.5 * Wo.sum(axis=1, dtype=f64)).astype(f32)
    consts = np.concatenate([np.tile(c16, QT), wsum, bo]).reshape(1, 64 + 2 * OUT)

    shared = {"consts": consts}
    in_maps = []
    for b in range(N_CORES):
        m = dict(shared)
        # token-major -> [p, k, t] with contraction index i = k*128+p
        pkt = np.ascontiguousarray(
            x[b].T.reshape(KC, P, S).transpose(1, 0, 2)
        )                                                                # [128,4,2048]
        x_hi = pkt.astype(f16)
        x_lo = (
            (pkt.astype(f64) - x_hi.astype(f64)) * R_SHIFT
        ).astype(f32).astype(e4m3)

        xhw = np.empty((P, KC, Q + 32), f16)
        xhw[:, :, :Q] = x_hi[:, :, 0:Q]
        xhw[:, :, Q:] = whl
        m["xhw"] = xhw
        m["xh"] = np.ascontiguousarray(x_hi[:, :, Q:])
        xlw = np.empty((P, KC, Q + 16), e4m3)
        xlw[:, :, :Q] = x_lo[:, :, 0:Q]
        xlw[:, :, Q:] = w8_pkj
        m["xlw"] = xlw
        m["xl"] = np.ascontiguousarray(x_lo[:, :, Q:])
        in_maps.append(m)
    return in_maps, bool(np.all(bo == 0.0)), bool(np.all(c16 == 0.0))


def run(inputs, trace=False, **run_kwargs):
    """Compile (cached), run on 8 cores, gather. Returns (out, BassKernelResults)."""
    from concourse.bass_utils import run_bass_kernel_spmd

    in_maps, bo_is_zero, c16_is_zero = _prep_inputs(**inputs)
    key = ("nc", bo_is_zero, c16_is_zero)
    if key not in _CACHE:
        _CACHE[key] = _build_nc(bo_is_zero, c16_is_zero)
    nc = _CACHE[key]
    _CACHE["nc"] = nc  # for test.py's TimelineSim hook

    try:
        res = run_bass_kernel_spmd(
            nc, in_maps, core_ids=list(range(N_CORES)), trace=trace, **run_kwargs
        )
    except Exception:
        # one retry for transient device wedges (NRT_TIMEOUT / unrecoverable)
        import time

        time.sleep(10)
        res = run_bass_kernel_spmd(
            nc, in_maps, core_ids=list(range(N_CORES)), trace=trace, **run_kwargs
        )
    out = np.stack([r["out"] for r in res.results], axis=0).astype(np.float32)
    return out, res


def kernel(x, Wi, bi, Wr, br, Wo, bo) -> np.ndarray:
    out, _ = run(dict(x=x, Wi=Wi, bi=bi, Wr=Wr, br=br, Wo=Wo, bo=bo))
    return out
